# revision 11
# baseline (speedup 1.0000x reference)
"""Trainium2 Bass kernel for nn_CrossSRA (spatial-reduction cross-attention).

Sharding (8 NeuronCores):
  - Batch-parallel for the main transformer path: core b owns batch b
    (q-projection, attention, output projection).
  - The spatial-reduction conv (768x768x8x8 weight, 151 MB fp32) is split by
    kernel-position row dy across the 8 cores: core j computes the partial
    conv output for ALL batches using conv_w[:, :, j, :] (9.4 MB bf16 per
    core instead of 75 MB replicated).  Partials are combined with an
    in-kernel ReduceScatter that simultaneously routes batch b's summed conv
    output to core b.

All matmuls run in bf16 with fp32 PSUM accumulation; layernorm/softmax
statistics stay in fp32.
"""

import numpy as np
import ml_dtypes

import concourse.bass as bass
import concourse.tile as tile
from concourse import bacc, mybir
from concourse.bass_utils import run_bass_kernel_spmd
from concourse.masks import make_identity

# problem shape (hardcoded per spec)
B = 8
N = 4096
C = 768
H = 8
DH = C // H            # 96
IMG = 64               # h = w = 64
SR = 8
KM = 64                # kv tokens after spatial reduction (8x8)
EPS = 1e-5
SCALE = DH ** -0.5

P = 128
CT = C // P            # 6 channel tiles
NCHUNK = 512
NCH = N // NCHUNK      # 8 column chunks

BF = mybir.dt.bfloat16
F32 = mybir.dt.float32
BF_NP = ml_dtypes.bfloat16

_CACHE: dict = {}


def _build_program():
    nc = bacc.Bacc("TRN2", target_bir_lowering=False, debug=False, num_devices=8)

    d_in = {}
    def din(name, shape, dt):
        d_in[name] = nc.dram_tensor(name, shape, dt, kind="ExternalInput").ap()
        return d_in[name]

    qxT = din("qxT", [C, N], BF)          # this batch's qx, transposed
    # all batches' kvx tokens with dy=core, grouped [c, dx, b, i, jj] so the
    # conv stationary operand is a contiguous 128-token slice per (ct, dx, pair)
    kvg = din("kvg", [C, SR * B * KM], BF)  # [768, 4096]
    cwT = din("cwT", [SR, C, C], BF)      # conv_w[o, c, dy=core, dx] -> [dx, c, o]
    kvb = din("kvb", [IMG, IMG], F32)     # this batch's kv_bias image
    qwT = din("qwT", [C, C], BF)
    kwT = din("kwT", [C, C], BF)
    vwT = din("vwT", [C, C], BF)
    pwT = din("pwT", [DH, H, C], BF)      # proj_w[o, 96h+d] -> [d, h, o]
    qb2 = din("qb2", [DH, H], F32)
    kb2 = din("kb2", [DH, H], F32)
    vb = din("vb", [C], F32)
    cb = din("cb", [C], F32)
    pb = din("pb", [C], F32)
    lnw = din("lnw", [C], F32)
    lnb = din("lnb", [C], F32)

    out = nc.dram_tensor("out", [N, C], F32, kind="ExternalOutput").ap()

    def bcast(vec_ap, parts):
        return bass.AP(tensor=vec_ap.tensor, offset=0, ap=[[0, parts], [1, C]])

    with tile.TileContext(nc) as tc:
        import contextlib
        stack = contextlib.ExitStack()
        with stack:
            consts = stack.enter_context(tc.tile_pool(name="consts", bufs=1))
            wpool = stack.enter_context(tc.tile_pool(name="weights", bufs=1))
            dram = stack.enter_context(tc.tile_pool(name="dram", bufs=1, space="DRAM"))

            # ---- constants ----
            ident = consts.tile([KM, KM], F32, tag="ident")
            make_identity(nc, ident[:])
            ones_sb = consts.tile([KM, KM], BF, tag="ones")
            nc.vector.memset(ones_sb[:], 1.0)
            eps_t = consts.tile([KM, 1], F32, tag="eps")
            nc.vector.memset(eps_t[:], EPS)

            vb_b = consts.tile([KM, C], F32, tag="vb")
            nc.sync.dma_start(vb_b[:], bcast(vb, KM))
            cb_b = consts.tile([KM, C], F32, tag="cb")
            nc.sync.dma_start(cb_b[:], bcast(cb, KM))
            lnw_b = consts.tile([KM, C], F32, tag="lnw")
            nc.sync.dma_start(lnw_b[:], bcast(lnw, KM))
            lnb_b = consts.tile([KM, C], F32, tag="lnb")
            nc.sync.dma_start(lnb_b[:], bcast(lnb, KM))
            pb_b = consts.tile([P, C], F32, tag="pb")
            nc.sync.dma_start(pb_b[:], bcast(pb, P))
            qb_sb = consts.tile([DH, H], F32, tag="qb")
            nc.sync.dma_start(qb_sb[:], qb2[:])
            kb_sb = consts.tile([DH, H], F32, tag="kb")
            nc.sync.dma_start(kb_sb[:], kb2[:])

            # attention bias: 4-point average of the bilinear resize (64->8)
            g4 = consts.tile([8, 8, 2, 2], F32, tag="g4")
            for dy in range(2):
                src = bass.AP(tensor=kvb.tensor, offset=(3 + dy) * IMG + 3,
                              ap=[[8 * IMG, 8], [8, 8], [1, 2]])
                nc.sync.dma_start(g4[:, :, dy, :], src)
            s4 = consts.tile([8, 8], F32, tag="s4")
            nc.vector.reduce_sum(s4[:], g4[:], axis=mybir.AxisListType.XY)
            s4q = consts.tile([8, 8], F32, tag="s4q")
            nc.scalar.mul(s4q[:], s4[:], 0.25)
            attnb = consts.tile([KM, 1], F32, tag="attnb")
            nc.sync.dma_start(attnb[:], s4q[:])  # [8p,8f] -> [64p,1f]

            # ---- weights ----
            def load_wT(src_ap, tag):
                t = wpool.tile([P, CT, C], BF, tag=tag)
                view = bass.AP(tensor=src_ap.tensor, offset=0,
                               ap=[[C, P], [P * C, CT], [1, C]])
                nc.sync.dma_start(t[:], view)
                return t

            qw_sb = load_wT(qwT, "qw")
            kw_sb = load_wT(kwT, "kw")
            vw_sb = load_wT(vwT, "vw")
            pw_sb = wpool.tile([DH, H, C], BF, tag="pw")
            nc.sync.dma_start(pw_sb[:], pwT[:])

            # collective bounce buffers
            partials = dram.tile([B * KM, C], F32, tag="partials")
            kvred = dram.tile([KM, C], F32, tag="kvred")

            # ================= conv phase (k-split over dy) =================
            with tc.tile_pool(name="convp", bufs=1) as cvpool, \
                 tc.tile_pool(name="cwstream", bufs=2) as cwpool, \
                 tc.tile_pool(name="cvpsum", bufs=1, space="PSUM") as cvps, \
                 tc.tile_pool(name="cvout", bufs=2) as cvo:
                kvx_sb = cvpool.tile([P, CT, SR, 512], BF, tag="kvx")
                view = bass.AP(tensor=kvg.tensor, offset=0,
                               ap=[[SR * 512, P], [P * SR * 512, CT], [1, SR * 512]])
                nc.sync.dma_start(
                    kvx_sb[:].rearrange("p ct dx t -> p ct (dx t)"), view)

                cps = [cvps.tile([P, C], F32, tag=f"cv{i}", name=f"cv{i}")
                       for i in range(4)]
                for dx in range(SR):
                    cw_t = cwpool.tile([P, CT, C], BF, tag="cw")
                    view = bass.AP(tensor=cwT.tensor, offset=dx * C * C,
                                   ap=[[C, P], [P * C, CT], [1, C]])
                    nc.sync.dma_start(cw_t[:], view)
                    for pr in range(4):  # batch pair (2pr, 2pr+1) stacked in M
                        pc = cps[pr]
                        for ct in range(CT):
                            lhsT = kvx_sb[:, ct, dx, pr * P:(pr + 1) * P]
                            for o0, osz in ((0, 512), (512, 256)):
                                nc.tensor.matmul(
                                    pc[:, o0:o0 + osz], lhsT,
                                    cw_t[:, ct, o0:o0 + osz],
                                    start=(dx == 0 and ct == 0),
                                    stop=(dx == SR - 1 and ct == CT - 1))
                for pr in range(4):
                    pt = cvo.tile([P, C], F32, tag="cvo")
                    nc.vector.tensor_copy(pt[:], cps[pr][:])
                    nc.sync.dma_start(partials[pr * P:(pr + 1) * P, :], pt[:])

            # ================= reduce-scatter =================
            nc.gpsimd.collective_compute(
                "ReduceScatter", mybir.AluOpType.add,
                replica_groups=[list(range(8))],
                ins=[partials.opt()], outs=[kvred.opt()])

            # ================= q projection (all chunks; covers RS latency) ==
            qpool = stack.enter_context(tc.tile_pool(name="qT", bufs=1))
            qxpool = stack.enter_context(tc.tile_pool(name="qx", bufs=3))
            qT_sb = qpool.tile([DH, H, N], BF, tag="qT")
            with tc.tile_pool(name="qpsum", bufs=2, space="PSUM") as qps:
                for ch in range(NCH):
                    qx_t = qxpool.tile([P, CT, NCHUNK], BF, tag="qx")
                    view = bass.AP(tensor=qxT.tensor, offset=ch * NCHUNK,
                                   ap=[[N, P], [P * N, CT], [1, NCHUNK]])
                    nc.sync.dma_start(qx_t[:], view)
                    for h in range(H):
                        pq = qps.tile([DH, NCHUNK], F32, tag="pq")
                        for ct in range(CT):
                            nc.tensor.matmul(pq[:], qw_sb[:, ct, h * DH:(h + 1) * DH],
                                             qx_t[:, ct, :],
                                             start=(ct == 0), stop=(ct == CT - 1))
                        nc.scalar.activation(qT_sb[:, h, ch * NCHUNK:(ch + 1) * NCHUNK],
                                             pq[:], mybir.ActivationFunctionType.Identity,
                                             bias=qb_sb[:, h:h + 1])

            # ================= kv epilogue =================
            kvpool = stack.enter_context(tc.tile_pool(name="kv", bufs=1))
            kvps_ctx = tc.tile_pool(name="kvpsum", bufs=2, space="PSUM")
            kvps = kvps_ctx.__enter__()
            kv_sb = kvpool.tile([KM, C], F32, tag="kv")
            nc.sync.dma_start(kv_sb[:], kvred[:])
            nc.vector.tensor_add(kv_sb[:], kv_sb[:], cb_b[:])
            # layernorm over channels
            BD = nc.vector.BN_STATS_DIM
            stats = kvpool.tile([KM, 3, BD], F32, tag="stats")
            kv_g = kv_sb[:].rearrange("p (g d) -> p g d", g=3)
            for g in range(3):
                nc.vector.bn_stats(stats[:, g, :], kv_g[:, g, :])
            mv = kvpool.tile([KM, nc.vector.BN_AGGR_DIM], F32, tag="mv")
            nc.vector.bn_aggr(mv[:], stats[:])
            nc.scalar.activation(mv[:, 1:2], mv[:, 1:2],
                                 mybir.ActivationFunctionType.Sqrt, bias=eps_t[:])
            nc.vector.reciprocal(mv[:, 1:2], mv[:, 1:2])
            nc.vector.tensor_scalar(kv_sb[:], kv_sb[:],
                                    scalar1=mv[:, 0:1], scalar2=mv[:, 1:2],
                                    op0=mybir.AluOpType.subtract,
                                    op1=mybir.AluOpType.mult)
            nc.vector.tensor_mul(kv_sb[:], kv_sb[:], lnw_b[:])
            nc.vector.tensor_add(kv_sb[:], kv_sb[:], lnb_b[:])

            # transpose kv -> kvT [c, m]
            kvT_sb = kvpool.tile([P, CT, KM], BF, tag="kvT")
            for ct in range(CT):
                ptr = kvps.tile([P, KM], F32, tag="ptr")
                nc.tensor.transpose(ptr[:], kv_sb[:, ct * P:(ct + 1) * P], ident[:])
                nc.scalar.activation(kvT_sb[:, ct, :], ptr[:],
                                     mybir.ActivationFunctionType.Identity)
            # k projection -> kT [d, h, m]
            kT_sb = kvpool.tile([DH, H, KM], BF, tag="kT")
            for h in range(H):
                pk = kvps.tile([DH, KM], F32, tag="pk")
                for ct in range(CT):
                    nc.tensor.matmul(pk[:], kw_sb[:, ct, h * DH:(h + 1) * DH],
                                     kvT_sb[:, ct, :],
                                     start=(ct == 0), stop=(ct == CT - 1))
                nc.scalar.activation(kT_sb[:, h, :], pk[:],
                                     mybir.ActivationFunctionType.Identity,
                                     bias=kb_sb[:, h:h + 1])
            # v projection -> v [m, c]
            pv1 = kvps.tile([KM, 512], F32, tag="pv1")
            pv2 = kvps.tile([KM, 256], F32, tag="pv2")
            for ct in range(CT):
                nc.tensor.matmul(pv1[:], kvT_sb[:, ct, :], vw_sb[:, ct, 0:512],
                                 start=(ct == 0), stop=(ct == CT - 1))
                nc.tensor.matmul(pv2[:], kvT_sb[:, ct, :], vw_sb[:, ct, 512:768],
                                 start=(ct == 0), stop=(ct == CT - 1))
            v_sb = kvpool.tile([KM, C], BF, tag="v")
            nc.vector.tensor_add(v_sb[:, 0:512], pv1[:], vb_b[:, 0:512])
            nc.vector.tensor_add(v_sb[:, 512:768], pv2[:], vb_b[:, 512:768])
            kvps_ctx.__exit__(None, None, None)

            # ================= attention + output proj, per chunk =================
            apool = stack.enter_context(tc.tile_pool(name="attn", bufs=2))
            xpool = stack.enter_context(tc.tile_pool(name="x", bufs=2))
            opool = stack.enter_context(tc.tile_pool(name="ob", bufs=2))
            pss = stack.enter_context(tc.tile_pool(name="pss", bufs=1, space="PSUM"))
            psd = stack.enter_context(tc.tile_pool(name="psd", bufs=1, space="PSUM"))
            psx = stack.enter_context(tc.tile_pool(name="psx", bufs=2, space="PSUM"))
            pso = stack.enter_context(tc.tile_pool(name="pso", bufs=2, space="PSUM"))

            for ch in range(NCH):
                x_sb = xpool.tile([DH, H, NCHUNK], BF, tag="x")
                for h in range(H):
                    ps_s = pss.tile([KM, NCHUNK], F32, tag="s")
                    nc.tensor.matmul(ps_s[:], kT_sb[:, h, :],
                                     qT_sb[:, h, ch * NCHUNK:(ch + 1) * NCHUNK],
                                     start=True, stop=True)
                    expS = apool.tile([KM, NCHUNK], BF, tag="e")
                    nc.scalar.activation(expS[:], ps_s[:],
                                         mybir.ActivationFunctionType.Exp,
                                         bias=attnb[:], scale=SCALE)
                    ps_d = psd.tile([KM, NCHUNK], F32, tag="d")
                    nc.tensor.matmul(ps_d[:], ones_sb[:], expS[:],
                                     start=True, stop=True)
                    rec = apool.tile([KM, NCHUNK], F32, tag="r")
                    nc.vector.reciprocal(rec[:], ps_d[:])
                    normP = apool.tile([KM, NCHUNK], BF, tag="n")
                    nc.vector.tensor_mul(normP[:], expS[:], rec[:])
                    ps_x = psx.tile([DH, NCHUNK], F32, tag="x")
                    nc.tensor.matmul(ps_x[:], v_sb[:, h * DH:(h + 1) * DH], normP[:],
                                     start=True, stop=True)
                    nc.scalar.activation(x_sb[:, h, :], ps_x[:],
                                         mybir.ActivationFunctionType.Identity)
                for nsub in range(4):
                    po1 = pso.tile([P, 512], F32, tag="o1")
                    po2 = pso.tile([P, 256], F32, tag="o2")
                    for h in range(H):
                        lx = x_sb[:, h, nsub * P:(nsub + 1) * P]
                        nc.tensor.matmul(po1[:], lx, pw_sb[:, h, 0:512],
                                         start=(h == 0), stop=(h == H - 1))
                        nc.tensor.matmul(po2[:], lx, pw_sb[:, h, 512:768],
                                         start=(h == 0), stop=(h == H - 1))
                    ob = opool.tile([P, C], F32, tag="ob")
                    nc.vector.tensor_add(ob[:, 0:512], po1[:], pb_b[:, 0:512])
                    nc.vector.tensor_add(ob[:, 512:768], po2[:], pb_b[:, 512:768])
                    row = ch * NCHUNK + nsub * P
                    nc.sync.dma_start(out[row:row + P, :], ob[:])

    nc.compile()
    return nc


def _prep_inputs(qx, kvx, kv_bias, q_w, q_b, k_w, k_b, v_w, v_b,
                 proj_w, proj_b, conv_w, conv_b, ln_w, ln_b):
    """Shard + lay out the full inputs for the 8 cores."""
    f32 = np.float32
    qwT = np.ascontiguousarray(q_w.T).astype(BF_NP)
    kwT = np.ascontiguousarray(k_w.T).astype(BF_NP)
    vwT = np.ascontiguousarray(v_w.T).astype(BF_NP)
    pwT = np.ascontiguousarray(proj_w.T.reshape(H, DH, C).transpose(1, 0, 2)).astype(BF_NP)
    qb2 = np.ascontiguousarray(q_b.reshape(H, DH).T).astype(f32)
    kb2 = np.ascontiguousarray(k_b.reshape(H, DH).T).astype(f32)

    # kvx token (512i + 64dy + 8jj + dx); core dy gets layout [ch, dx, b, i, jj]
    kv6 = kvx.reshape(B, 8, 8, 8, 8, C)
    in_maps = []
    for c in range(8):
        kvg = np.ascontiguousarray(
            kv6[:, :, c].transpose(4, 3, 0, 1, 2).reshape(C, SR * B * KM)
        ).astype(BF_NP)
        cwT = np.ascontiguousarray(conv_w[:, :, c, :].transpose(2, 1, 0)).astype(BF_NP)
        in_maps.append({
            "qxT": np.ascontiguousarray(qx[c].T).astype(BF_NP),
            "kvg": kvg,
            "cwT": cwT,
            "kvb": np.ascontiguousarray(kv_bias[c, 0]).astype(f32),
            "qwT": qwT, "kwT": kwT, "vwT": vwT, "pwT": pwT,
            "qb2": qb2, "kb2": kb2,
            "vb": v_b.astype(f32), "cb": conv_b.astype(f32),
            "pb": proj_b.astype(f32),
            "lnw": ln_w.astype(f32), "lnb": ln_b.astype(f32),
        })
    return in_maps


def _run(inputs: dict, trace: bool = False):
    if "nc" not in _CACHE:
        _CACHE["nc"] = _build_program()
    nc = _CACHE["nc"]
    in_maps = _prep_inputs(
        qx=np.asarray(inputs["qx"]), kvx=np.asarray(inputs["kvx"]),
        kv_bias=np.asarray(inputs["kv_bias"]),
        q_w=np.asarray(inputs["q_w"]), q_b=np.asarray(inputs["q_b"]),
        k_w=np.asarray(inputs["k_w"]), k_b=np.asarray(inputs["k_b"]),
        v_w=np.asarray(inputs["v_w"]), v_b=np.asarray(inputs["v_b"]),
        proj_w=np.asarray(inputs["proj_w"]), proj_b=np.asarray(inputs["proj_b"]),
        conv_w=np.asarray(inputs["conv_w"]), conv_b=np.asarray(inputs["conv_b"]),
        ln_w=np.asarray(inputs["ln_w"]), ln_b=np.asarray(inputs["ln_b"]))
    res = run_bass_kernel_spmd(nc, in_maps, core_ids=list(range(8)), trace=trace)
    full = np.stack([res.results[c]["out"] for c in range(8)], axis=0)
    return full.astype(np.float32), res


def kernel(**inputs) -> np.ndarray:
    full, _ = _run(inputs, trace=False)
    return full


# revision 13
# speedup vs baseline: 1.2498x; 1.2498x over previous
"""Trainium2 Bass kernel for nn_CrossSRA (spatial-reduction cross-attention).

Sharding (8 NeuronCores):
  - Batch-parallel for the main transformer path: core b owns batch b
    (q-projection, attention, output projection).
  - The spatial-reduction conv (768x768x8x8 weight, 151 MB fp32) is split by
    kernel-position row dy across the 8 cores: core j computes the partial
    conv output for ALL batches using conv_w[:, :, j, :] (9.4 MB bf16 per
    core instead of 75 MB replicated).  Partials are combined with an
    in-kernel ReduceScatter that simultaneously routes batch b's summed conv
    output to core b.

All matmuls run in bf16 with fp32 PSUM accumulation; layernorm/softmax
statistics stay in fp32.
"""

import numpy as np
import ml_dtypes

import concourse.bass as bass
import concourse.tile as tile
from concourse import bacc, mybir
from concourse.bass_utils import run_bass_kernel_spmd
from concourse.masks import make_identity

# problem shape (hardcoded per spec)
B = 8
N = 4096
C = 768
H = 8
DH = C // H            # 96
IMG = 64               # h = w = 64
SR = 8
KM = 64                # kv tokens after spatial reduction (8x8)
EPS = 1e-5
SCALE = DH ** -0.5

P = 128
CT = C // P            # 6 channel tiles
NCHUNK = 512
NCH = N // NCHUNK      # 8 column chunks

BF = mybir.dt.bfloat16
F32 = mybir.dt.float32
BF_NP = ml_dtypes.bfloat16

_CACHE: dict = {}


def _build_program():
    nc = bacc.Bacc("TRN2", target_bir_lowering=False, debug=False, num_devices=8)

    d_in = {}
    def din(name, shape, dt):
        d_in[name] = nc.dram_tensor(name, shape, dt, kind="ExternalInput").ap()
        return d_in[name]

    qxT = din("qxT", [C, N], BF)          # this batch's qx, transposed
    # all batches' kvx tokens with dy=core, grouped [c, dx, b, i, jj] so the
    # conv stationary operand is a contiguous 128-token slice per (ct, dx, pair)
    kvg = din("kvg", [C, SR * B * KM], BF)  # [768, 4096]
    cwT = din("cwT", [SR, C, C], BF)      # conv_w[o, c, dy=core, dx] -> [dx, c, o]
    kvb = din("kvb", [IMG, IMG], F32)     # this batch's kv_bias image
    qwT = din("qwT", [C, C], BF)
    kwT = din("kwT", [C, C], BF)
    vwT = din("vwT", [C, C], BF)
    pwT = din("pwT", [DH, H, C], BF)      # proj_w[o, 96h+d] -> [d, h, o]
    qb2 = din("qb2", [DH, H], F32)
    kb2 = din("kb2", [DH, H], F32)
    vb = din("vb", [C], F32)
    cb = din("cb", [C], F32)
    pb = din("pb", [C], F32)
    lnw = din("lnw", [C], F32)
    lnb = din("lnb", [C], F32)

    out = nc.dram_tensor("out", [N, C], F32, kind="ExternalOutput").ap()

    def bcast(vec_ap, parts):
        return bass.AP(tensor=vec_ap.tensor, offset=0, ap=[[0, parts], [1, C]])

    with tile.TileContext(nc) as tc:
        import contextlib
        stack = contextlib.ExitStack()
        with stack:
            consts = stack.enter_context(tc.tile_pool(name="consts", bufs=1))
            wpool = stack.enter_context(tc.tile_pool(name="weights", bufs=1))
            dram = stack.enter_context(tc.tile_pool(name="dram", bufs=1, space="DRAM"))

            # ---- constants ----
            ident = consts.tile([KM, KM], F32, tag="ident")
            make_identity(nc, ident[:])
            ones_sb = consts.tile([KM, KM], BF, tag="ones")
            nc.vector.memset(ones_sb[:], 1.0)
            eps_t = consts.tile([KM, 1], F32, tag="eps")
            nc.vector.memset(eps_t[:], EPS)

            vb_b = consts.tile([KM, C], F32, tag="vb")
            nc.sync.dma_start(vb_b[:], bcast(vb, KM))
            cb_b = consts.tile([KM, C], F32, tag="cb")
            nc.sync.dma_start(cb_b[:], bcast(cb, KM))
            lnw_b = consts.tile([KM, C], F32, tag="lnw")
            nc.sync.dma_start(lnw_b[:], bcast(lnw, KM))
            lnb_b = consts.tile([KM, C], F32, tag="lnb")
            nc.sync.dma_start(lnb_b[:], bcast(lnb, KM))
            pb_b = consts.tile([P, C], F32, tag="pb")
            nc.sync.dma_start(pb_b[:], bcast(pb, P))
            qb_sb = consts.tile([DH, H], F32, tag="qb")
            nc.sync.dma_start(qb_sb[:], qb2[:])
            kb_sb = consts.tile([DH, H], F32, tag="kb")
            nc.sync.dma_start(kb_sb[:], kb2[:])

            # attention bias: 4-point average of the bilinear resize (64->8)
            g4 = consts.tile([8, 8, 2, 2], F32, tag="g4")
            for dy in range(2):
                src = bass.AP(tensor=kvb.tensor, offset=(3 + dy) * IMG + 3,
                              ap=[[8 * IMG, 8], [8, 8], [1, 2]])
                nc.sync.dma_start(g4[:, :, dy, :], src)
            s4 = consts.tile([8, 8], F32, tag="s4")
            nc.vector.reduce_sum(s4[:], g4[:], axis=mybir.AxisListType.XY)
            s4q = consts.tile([8, 8], F32, tag="s4q")
            nc.scalar.mul(s4q[:], s4[:], 0.25)
            attnb = consts.tile([KM, 1], F32, tag="attnb")
            nc.sync.dma_start(attnb[:], s4q[:])  # [8p,8f] -> [64p,1f]

            # ---- weights ----
            def load_wT(src_ap, tag):
                t = wpool.tile([P, CT, C], BF, tag=tag)
                view = bass.AP(tensor=src_ap.tensor, offset=0,
                               ap=[[C, P], [P * C, CT], [1, C]])
                nc.sync.dma_start(t[:], view)
                return t

            qw_sb = load_wT(qwT, "qw")
            kw_sb = load_wT(kwT, "kw")
            vw_sb = load_wT(vwT, "vw")
            pw_sb = wpool.tile([DH, H, C], BF, tag="pw")
            nc.sync.dma_start(pw_sb[:], pwT[:])

            # collective bounce buffers
            partials = dram.tile([B * KM, C], F32, tag="partials")
            kvred = dram.tile([KM, C], F32, tag="kvred")

            # ================= conv phase (k-split over dy) =================
            with tc.tile_pool(name="convp", bufs=1) as cvpool, \
                 tc.tile_pool(name="cwstream", bufs=2) as cwpool, \
                 tc.tile_pool(name="cvpsum", bufs=1, space="PSUM") as cvps, \
                 tc.tile_pool(name="cvout", bufs=2) as cvo:
                kvx_sb = cvpool.tile([P, CT, SR, 512], BF, tag="kvx")
                view = bass.AP(tensor=kvg.tensor, offset=0,
                               ap=[[SR * 512, P], [P * SR * 512, CT], [1, SR * 512]])
                nc.sync.dma_start(
                    kvx_sb[:].rearrange("p ct dx t -> p ct (dx t)"), view)

                cps = [cvps.tile([P, C], F32, tag=f"cv{i}", name=f"cv{i}")
                       for i in range(4)]
                for dx in range(SR):
                    cw_t = cwpool.tile([P, CT, C], BF, tag="cw")
                    view = bass.AP(tensor=cwT.tensor, offset=dx * C * C,
                                   ap=[[C, P], [P * C, CT], [1, C]])
                    nc.sync.dma_start(cw_t[:], view)
                    for pr in range(4):  # batch pair (2pr, 2pr+1) stacked in M
                        pc = cps[pr]
                        for ct in range(CT):
                            lhsT = kvx_sb[:, ct, dx, pr * P:(pr + 1) * P]
                            for o0, osz in ((0, 512), (512, 256)):
                                nc.tensor.matmul(
                                    pc[:, o0:o0 + osz], lhsT,
                                    cw_t[:, ct, o0:o0 + osz],
                                    start=(dx == 0 and ct == 0),
                                    stop=(dx == SR - 1 and ct == CT - 1))
                for pr in range(4):
                    pt = cvo.tile([P, C], F32, tag="cvo")
                    nc.vector.tensor_copy(pt[:], cps[pr][:])
                    nc.sync.dma_start(partials[pr * P:(pr + 1) * P, :], pt[:])

            # ================= reduce-scatter =================
            nc.gpsimd.collective_compute(
                "ReduceScatter", mybir.AluOpType.add,
                replica_groups=[list(range(8))],
                ins=[partials.opt()], outs=[kvred.opt()])

            # ================= q projection (all chunks; covers RS latency) ==
            qpool = stack.enter_context(tc.tile_pool(name="qT", bufs=1))
            qxpool = stack.enter_context(tc.tile_pool(name="qx", bufs=3))
            qT_sb = qpool.tile([DH, H, N], BF, tag="qT")
            with tc.tile_pool(name="qpsum", bufs=2, space="PSUM") as qps:
                for ch in range(NCH):
                    qx_t = qxpool.tile([P, CT, NCHUNK], BF, tag="qx")
                    view = bass.AP(tensor=qxT.tensor, offset=ch * NCHUNK,
                                   ap=[[N, P], [P * N, CT], [1, NCHUNK]])
                    nc.sync.dma_start(qx_t[:], view)
                    for h in range(H):
                        pq = qps.tile([DH, NCHUNK], F32, tag="pq")
                        for ct in range(CT):
                            nc.tensor.matmul(pq[:], qw_sb[:, ct, h * DH:(h + 1) * DH],
                                             qx_t[:, ct, :],
                                             start=(ct == 0), stop=(ct == CT - 1))
                        nc.scalar.activation(qT_sb[:, h, ch * NCHUNK:(ch + 1) * NCHUNK],
                                             pq[:], mybir.ActivationFunctionType.Identity,
                                             bias=qb_sb[:, h:h + 1])

            # ================= kv epilogue =================
            kvpool = stack.enter_context(tc.tile_pool(name="kv", bufs=1))
            kvps_ctx = tc.tile_pool(name="kvpsum", bufs=2, space="PSUM")
            kvps = kvps_ctx.__enter__()
            kv_sb = kvpool.tile([KM, C], F32, tag="kv")
            nc.sync.dma_start(kv_sb[:], kvred[:])
            nc.vector.tensor_add(kv_sb[:], kv_sb[:], cb_b[:])
            # layernorm over channels
            BD = nc.vector.BN_STATS_DIM
            stats = kvpool.tile([KM, 3, BD], F32, tag="stats")
            kv_g = kv_sb[:].rearrange("p (g d) -> p g d", g=3)
            for g in range(3):
                nc.vector.bn_stats(stats[:, g, :], kv_g[:, g, :])
            mv = kvpool.tile([KM, nc.vector.BN_AGGR_DIM], F32, tag="mv")
            nc.vector.bn_aggr(mv[:], stats[:])
            nc.scalar.activation(mv[:, 1:2], mv[:, 1:2],
                                 mybir.ActivationFunctionType.Sqrt, bias=eps_t[:])
            nc.vector.reciprocal(mv[:, 1:2], mv[:, 1:2])
            nc.vector.tensor_scalar(kv_sb[:], kv_sb[:],
                                    scalar1=mv[:, 0:1], scalar2=mv[:, 1:2],
                                    op0=mybir.AluOpType.subtract,
                                    op1=mybir.AluOpType.mult)
            nc.vector.tensor_mul(kv_sb[:], kv_sb[:], lnw_b[:])
            nc.vector.tensor_add(kv_sb[:], kv_sb[:], lnb_b[:])

            # transpose kv -> kvT [c, m]
            kvT_sb = kvpool.tile([P, CT, KM], BF, tag="kvT")
            for ct in range(CT):
                ptr = kvps.tile([P, KM], F32, tag="ptr")
                nc.tensor.transpose(ptr[:], kv_sb[:, ct * P:(ct + 1) * P], ident[:])
                nc.scalar.activation(kvT_sb[:, ct, :], ptr[:],
                                     mybir.ActivationFunctionType.Identity)
            # k projection -> kT [d, h, m]
            kT_sb = kvpool.tile([DH, H, KM], BF, tag="kT")
            for h in range(H):
                pk = kvps.tile([DH, KM], F32, tag="pk")
                for ct in range(CT):
                    nc.tensor.matmul(pk[:], kw_sb[:, ct, h * DH:(h + 1) * DH],
                                     kvT_sb[:, ct, :],
                                     start=(ct == 0), stop=(ct == CT - 1))
                nc.scalar.activation(kT_sb[:, h, :], pk[:],
                                     mybir.ActivationFunctionType.Identity,
                                     bias=kb_sb[:, h:h + 1])
            # v projection -> v [m, c]
            pv1 = kvps.tile([KM, 512], F32, tag="pv1")
            pv2 = kvps.tile([KM, 256], F32, tag="pv2")
            for ct in range(CT):
                nc.tensor.matmul(pv1[:], kvT_sb[:, ct, :], vw_sb[:, ct, 0:512],
                                 start=(ct == 0), stop=(ct == CT - 1))
                nc.tensor.matmul(pv2[:], kvT_sb[:, ct, :], vw_sb[:, ct, 512:768],
                                 start=(ct == 0), stop=(ct == CT - 1))
            v_sb = kvpool.tile([KM, C], BF, tag="v")
            nc.vector.tensor_add(v_sb[:, 0:512], pv1[:], vb_b[:, 0:512])
            nc.vector.tensor_add(v_sb[:, 512:768], pv2[:], vb_b[:, 512:768])
            kvps_ctx.__exit__(None, None, None)

            # ================= attention + output proj, per chunk =================
            apool = stack.enter_context(tc.tile_pool(name="attn", bufs=2))
            xpool = stack.enter_context(tc.tile_pool(name="x", bufs=2))
            opool = stack.enter_context(tc.tile_pool(name="ob", bufs=2))
            pss = stack.enter_context(tc.tile_pool(name="pss", bufs=2, space="PSUM"))
            psd = stack.enter_context(tc.tile_pool(name="psd", bufs=1, space="PSUM"))
            psx = stack.enter_context(tc.tile_pool(name="psx", bufs=1, space="PSUM"))
            pso = stack.enter_context(tc.tile_pool(name="pso", bufs=2, space="PSUM"))

            for ch in range(NCH):
                x_sb = xpool.tile([DH, H, NCHUNK], BF, tag="x")
                for h in range(H):
                    ps_s = pss.tile([KM, NCHUNK], F32, tag="s")
                    nc.tensor.matmul(ps_s[:], kT_sb[:, h, :],
                                     qT_sb[:, h, ch * NCHUNK:(ch + 1) * NCHUNK],
                                     start=True, stop=True)
                    expS = apool.tile([KM, NCHUNK], BF, tag="e")
                    nc.scalar.activation(expS[:], ps_s[:],
                                         mybir.ActivationFunctionType.Exp,
                                         bias=attnb[:], scale=SCALE)
                    ps_d = psd.tile([KM, NCHUNK], F32, tag="d")
                    nc.tensor.matmul(ps_d[:], ones_sb[:], expS[:],
                                     start=True, stop=True)
                    rec = apool.tile([KM, NCHUNK], F32, tag="r")
                    nc.vector.reciprocal_approx_fast(rec[:], ps_d[:])
                    normP = apool.tile([KM, NCHUNK], BF, tag="n")
                    nc.vector.tensor_mul(normP[:], expS[:], rec[:])
                    ps_x = psx.tile([DH, NCHUNK], F32, tag="x")
                    nc.tensor.matmul(ps_x[:], v_sb[:, h * DH:(h + 1) * DH], normP[:],
                                     start=True, stop=True)
                    nc.scalar.activation(x_sb[:, h, :], ps_x[:],
                                         mybir.ActivationFunctionType.Identity)
                for nsub in range(4):
                    po1 = pso.tile([P, 512], F32, tag="o1")
                    po2 = pso.tile([P, 256], F32, tag="o2")
                    for h in range(H):
                        lx = x_sb[:, h, nsub * P:(nsub + 1) * P]
                        nc.tensor.matmul(po1[:], lx, pw_sb[:, h, 0:512],
                                         start=(h == 0), stop=(h == H - 1))
                        nc.tensor.matmul(po2[:], lx, pw_sb[:, h, 512:768],
                                         start=(h == 0), stop=(h == H - 1))
                    ob = opool.tile([P, C], F32, tag="ob")
                    nc.vector.tensor_add(ob[:, 0:512], po1[:], pb_b[:, 0:512])
                    nc.vector.tensor_add(ob[:, 512:768], po2[:], pb_b[:, 512:768])
                    row = ch * NCHUNK + nsub * P
                    nc.sync.dma_start(out[row:row + P, :], ob[:])

    nc.compile()
    return nc


def _prep_inputs(qx, kvx, kv_bias, q_w, q_b, k_w, k_b, v_w, v_b,
                 proj_w, proj_b, conv_w, conv_b, ln_w, ln_b):
    """Shard + lay out the full inputs for the 8 cores."""
    f32 = np.float32
    qwT = np.ascontiguousarray(q_w.T).astype(BF_NP)
    kwT = np.ascontiguousarray(k_w.T).astype(BF_NP)
    vwT = np.ascontiguousarray(v_w.T).astype(BF_NP)
    pwT = np.ascontiguousarray(proj_w.T.reshape(H, DH, C).transpose(1, 0, 2)).astype(BF_NP)
    qb2 = np.ascontiguousarray(q_b.reshape(H, DH).T).astype(f32)
    kb2 = np.ascontiguousarray(k_b.reshape(H, DH).T).astype(f32)

    # kvx token (512i + 64dy + 8jj + dx); core dy gets layout [ch, dx, b, i, jj]
    kv6 = kvx.reshape(B, 8, 8, 8, 8, C)
    in_maps = []
    for c in range(8):
        kvg = np.ascontiguousarray(
            kv6[:, :, c].transpose(4, 3, 0, 1, 2).reshape(C, SR * B * KM)
        ).astype(BF_NP)
        cwT = np.ascontiguousarray(conv_w[:, :, c, :].transpose(2, 1, 0)).astype(BF_NP)
        in_maps.append({
            "qxT": np.ascontiguousarray(qx[c].T).astype(BF_NP),
            "kvg": kvg,
            "cwT": cwT,
            "kvb": np.ascontiguousarray(kv_bias[c, 0]).astype(f32),
            "qwT": qwT, "kwT": kwT, "vwT": vwT, "pwT": pwT,
            "qb2": qb2, "kb2": kb2,
            "vb": v_b.astype(f32), "cb": conv_b.astype(f32),
            "pb": proj_b.astype(f32),
            "lnw": ln_w.astype(f32), "lnb": ln_b.astype(f32),
        })
    return in_maps


def _run(inputs: dict, trace: bool = False):
    if "nc" not in _CACHE:
        _CACHE["nc"] = _build_program()
    nc = _CACHE["nc"]
    in_maps = _prep_inputs(
        qx=np.asarray(inputs["qx"]), kvx=np.asarray(inputs["kvx"]),
        kv_bias=np.asarray(inputs["kv_bias"]),
        q_w=np.asarray(inputs["q_w"]), q_b=np.asarray(inputs["q_b"]),
        k_w=np.asarray(inputs["k_w"]), k_b=np.asarray(inputs["k_b"]),
        v_w=np.asarray(inputs["v_w"]), v_b=np.asarray(inputs["v_b"]),
        proj_w=np.asarray(inputs["proj_w"]), proj_b=np.asarray(inputs["proj_b"]),
        conv_w=np.asarray(inputs["conv_w"]), conv_b=np.asarray(inputs["conv_b"]),
        ln_w=np.asarray(inputs["ln_w"]), ln_b=np.asarray(inputs["ln_b"]))
    res = run_bass_kernel_spmd(nc, in_maps, core_ids=list(range(8)), trace=trace)
    full = np.stack([res.results[c]["out"] for c in range(8)], axis=0)
    return full.astype(np.float32), res


def kernel(**inputs) -> np.ndarray:
    full, _ = _run(inputs, trace=False)
    return full
